# revision 11
# baseline (speedup 1.0000x reference)
"""AdditiveAttention distributed Bass kernel for 8 TRN2 NeuronCores.

Data-parallel over batch: B=8 samples -> 1 per core. Weights replicated.

Per-core math (S=2048, D=1024, H=16, HD=64):
  q = X @ W_qv + b_qv                 ; v = q
  k = X @ W_k + b_k
  alphas = softmax_h((q @ Wq_s + bq_s) * s)       s = 1/sqrt(HD)
  gq[d]  = sum_s alphas[s, h(d)] * q[s, d]        h(d) = d // 64
  p = k * gq                                       (broadcast over s)
  betas  = softmax_h((p @ Wk_s + bk_s) * s)
  gk[d]  = sum_s betas[s, h(d)] * p[s, d] = gq[d] * sum_s betas[s,h(d)] k[s,d]
  u = q * gk
  out = q + u @ W_r + b_r

Layout strategy: activations kept TRANSPOSED (qT[d, s], d on partitions) so the
three big matmuls use the naturally-stored weights [c, d] as stationary lhsT and
XT[c, s] as moving rhs.  gq/gk are computed without materializing natural-layout
activations via:  gq_raw = W_qv^T @ (X^T @ alphas) + b_qv * colsum(alphas),
where X^T @ alphas streams natural X tiles from DRAM as lhsT.  The final
residual "q +" is folded into PSUM by PE-transposing qT tiles into the output
accumulator before the u @ W_r matmuls (start=False) run.
Matmuls run in float32r (full-rate PE mode, f32 storage).
"""

import math
import os
from contextlib import ExitStack

import numpy as np

B, S, D, H = 8, 2048, 1024, 16
HD = D // H
SCALE = 1.0 / math.sqrt(HD)
NCORES = 8
P = 128
NDB = D // P      # 8 d-blocks
NSB = S // P      # 16 s-blocks
NCC = D // P      # 8 contraction chunks
SH = 512          # psum free width for big matmuls
NSH = S // SH     # 4
NDH = D // SH     # 2

_CACHE = {}


def _build():
    import concourse.bacc as bacc
    import concourse.tile as tile
    import concourse.mybir as mybir

    f32 = mybir.dt.float32
    f32r = mybir.dt.float32r
    AF = mybir.ActivationFunctionType
    ALU = mybir.AluOpType

    nc = bacc.Bacc("TRN2", target_bir_lowering=False, debug=False,
                   num_devices=NCORES)

    X = nc.dram_tensor("X", [S, D], f32, kind="ExternalInput").ap()
    W_qv = nc.dram_tensor("W_qv", [D, D], f32, kind="ExternalInput").ap()
    b_qv = nc.dram_tensor("b_qv", [D], f32, kind="ExternalInput").ap()
    W_k = nc.dram_tensor("W_k", [D, D], f32, kind="ExternalInput").ap()
    b_k = nc.dram_tensor("b_k", [D], f32, kind="ExternalInput").ap()
    Wq_s = nc.dram_tensor("Wq_s", [D, H], f32, kind="ExternalInput").ap()
    bq_s = nc.dram_tensor("bq_s", [H], f32, kind="ExternalInput").ap()
    Wk_s = nc.dram_tensor("Wk_s", [D, H], f32, kind="ExternalInput").ap()
    bk_s = nc.dram_tensor("bk_s", [H], f32, kind="ExternalInput").ap()
    W_r = nc.dram_tensor("W_r", [D, D], f32, kind="ExternalInput").ap()
    b_r = nc.dram_tensor("b_r", [D], f32, kind="ExternalInput").ap()
    OUT = nc.dram_tensor("out", [S, D], f32, kind="ExternalOutput").ap()

    def r(ap):
        return ap.bitcast(f32r)

    with tile.TileContext(nc) as tc, ExitStack() as ctx:
        sbp = ctx.enter_context(tc.tile_pool(name="sbp", bufs=1))
        psp = ctx.enter_context(tc.tile_pool(name="psp", bufs=1, space="PSUM"))

        class _P:
            def __init__(self, pool, bufs):
                self.pool, self.bufs = pool, bufs
            def tile(self, shape, dt_, tag):
                return self.pool.tile(shape, dt_, tag=tag, bufs=self.bufs,
                                      name=tag)

        big = _P(sbp, 1)
        qtp = _P(sbp, 1)
        wp = _P(sbp, 2)
        xp = _P(sbp, 2)
        ptp = _P(sbp, 1)
        sm = _P(sbp, 1)
        outp = _P(sbp, 2)
        wrp = _P(sbp, 1)
        psb = _P(psp, 3)
        pst = _P(psp, 2)
        pss = _P(psp, 3)

        # ---------- constants ----------
        ones128 = sm.tile([P, P], f32, tag="ones128")
        nc.gpsimd.memset(ones128[:], 1.0)
        eye = sm.tile([P, P], f32, tag="eye")
        nc.gpsimd.affine_select(eye[:], ones128[:], pattern=[[1, P]],
                                compare_op=ALU.is_equal, fill=0.0,
                                base=0, channel_multiplier=-1)
        eye_r = sm.tile([P, P], f32r, tag="eye_r")
        nc.vector.tensor_copy(eye_r[:], eye[:])
        ones_row = sm.tile([1, P], f32r, tag="ones_row")
        nc.vector.tensor_copy(ones_row[:1, :], ones128[:1, :])
        ones_col = sm.tile([P, 1], f32r, tag="ones_col")
        nc.vector.tensor_copy(ones_col[:], ones128[:, 0:1])

        # ---------- small weights / biases ----------
        bqv_pp = sm.tile([P, NDB], f32, tag="bqv_pp")
        nc.sync.dma_start(bqv_pp[:], b_qv.rearrange("(j p) -> p j", p=P))
        bk_pp = sm.tile([P, NDB], f32, tag="bk_pp")
        nc.sync.dma_start(bk_pp[:], b_k.rearrange("(j p) -> p j", p=P))
        bqv_row = sm.tile([1, D], f32r, tag="bqv_row")
        nc.sync.dma_start(bqv_row[:], b_qv.unsqueeze(0).bitcast(f32r))
        bk_row = sm.tile([1, D], f32r, tag="bk_row")
        nc.sync.dma_start(bk_row[:], b_k.unsqueeze(0).bitcast(f32r))
        br_row = sm.tile([1, D], f32r, tag="br_row")
        nc.sync.dma_start(br_row[:], b_r.unsqueeze(0).bitcast(f32r))
        bqs_row = sm.tile([1, H], f32r, tag="bqs_row")
        nc.sync.dma_start(bqs_row[:], bq_s.unsqueeze(0).bitcast(f32r))
        bks_row = sm.tile([1, H], f32r, tag="bks_row")
        nc.sync.dma_start(bks_row[:], bk_s.unsqueeze(0).bitcast(f32r))
        wqs_sb = sm.tile([P, NDB * H], f32r, tag="wqs_sb")
        nc.sync.dma_start(wqs_sb[:].rearrange("p (j h) -> p j h", j=NDB),
                          Wq_s.bitcast(f32r).rearrange("(j p) h -> p j h", p=P))
        wks_sb = sm.tile([P, NDB * H], f32r, tag="wks_sb")
        nc.sync.dma_start(wks_sb[:].rearrange("p (j h) -> p j h", j=NDB),
                          Wk_s.bitcast(f32r).rearrange("(j p) h -> p j h", p=P))

        # ---------- pre-allocate all small persistent tiles ----------
        aE = sm.tile([P, NSB * H], f32, tag="aE")
        Za = sm.tile([P, NSB], f32, tag="Za")
        rZa = sm.tile([P, NSB], f32, tag="rZa")
        alpha = sm.tile([P, NSB * H], f32r, tag="alpha")
        bE = sm.tile([P, NSB * H], f32, tag="bE")
        Zb = sm.tile([P, NSB], f32, tag="Zb")
        rZb = sm.tile([P, NSB], f32, tag="rZb")
        beta = sm.tile([P, NSB * H], f32r, tag="beta")
        Asb_q = sm.tile([P, NCC * H], f32r, tag="Asbq")
        Ssb_q = sm.tile([1, H], f32r, tag="Ssbq")
        Asb_k = sm.tile([P, NCC * H], f32r, tag="Asbk")
        Ssb_k = sm.tile([1, H], f32r, tag="Ssbk")
        gq = sm.tile([P, NDB], f32, tag="gq")
        gkd = sm.tile([P, NDB], f32, tag="gkd")
        gk = sm.tile([P, NDB], f32, tag="gk")

        # ---------- phase 1: XT via PE transpose ----------
        xt = big.tile([P, NCC * S], f32r, tag="bigA")   # chunk cc at cols cc*S
        for si in range(NSB):
            xn = xp.tile([P, D], f32r, tag="xn")
            nc.sync.dma_start(xn[:], X[si * P:(si + 1) * P, :].bitcast(f32r))
            for cc in range(NCC):
                tp = pst.tile([P, P], f32, tag="tp")
                nc.tensor.transpose(tp[:].bitcast(f32r), xn[:, cc * P:(cc + 1) * P], eye_r[:])
                dst = xt[:, cc * S + si * P: cc * S + si * P + P]
                if (si * NCC + cc) % 2 == 0:
                    nc.scalar.copy(dst, tp[:])
                else:
                    nc.vector.tensor_copy(dst, tp[:])

        # ---------- phase 2: qT = (X @ W_qv)^T + b_qv ----------
        qt = qtp.tile([P, NDB * S], f32r, tag="qt")     # d-block j at cols j*S
        for j in range(NDB):
            wq = wp.tile([P, D], f32r, tag="w")
            nc.sync.dma_start(
                wq[:].rearrange("p (c m) -> p c m", c=NCC),
                W_qv.bitcast(f32r)[:, j * P:(j + 1) * P].rearrange("(c p) m -> p c m", p=P))
            for sh in range(NSH):
                ps = psb.tile([P, SH], f32, tag="big")
                for cc in range(NCC):
                    nc.tensor.matmul(
                        ps[:], r(wq[:, cc * P:(cc + 1) * P]),
                        xt[:, cc * S + sh * SH: cc * S + sh * SH + SH],
                        start=(cc == 0), stop=(cc == NCC - 1))
                nc.vector.tensor_scalar(
                    qt[:, j * S + sh * SH: j * S + sh * SH + SH], ps[:],
                    bqv_pp[:, j:j + 1], None, ALU.add)

        # ---------- phase 3: alphas ----------
        logA = pss.tile([P, NSB * H], f32, tag="small")
        for sb in range(NSB):
            for j in range(NDB):
                nc.tensor.matmul(
                    logA[:, sb * H:(sb + 1) * H],
                    qt[:, j * S + sb * P: j * S + sb * P + P],
                    r(wqs_sb[:, j * H:(j + 1) * H]),
                    start=(sb == 0 and j == 0), stop=False)
            nc.tensor.matmul(
                logA[:, sb * H:(sb + 1) * H],
                ones_row[:1, :], bqs_row[:1, :],
                start=False, stop=(sb == NSB - 1))
        for sb in range(NSB):
            nc.scalar.activation(
                aE[:, sb * H:(sb + 1) * H], logA[:, sb * H:(sb + 1) * H],
                AF.Exp, bias=0.0, scale=SCALE, accum_out=Za[:, sb:sb + 1])
        nc.vector.reciprocal(rZa[:], Za[:])
        for sb in range(NSB):
            nc.vector.tensor_scalar(
                alpha[:, sb * H:(sb + 1) * H], aE[:, sb * H:(sb + 1) * H],
                rZa[:, sb:sb + 1], None, ALU.mult)

        # ---------- phase 4: gq ----------
        def weighted_sum(weights_sb, W_big, b_row, Asb, Ssb, g):
            """graw[d, h] = sum_s weights[s, h] * (X @ W_big + b)[s, d]   (packed)"""
            Aps = pss.tile([P, NCC * H], f32, tag="small")
            Sps = pss.tile([1, H], f32, tag="small")
            for si in range(NSB):
                xn = xp.tile([P, D], f32r, tag="xn")
                nc.sync.dma_start(xn[:], X[si * P:(si + 1) * P, :].bitcast(f32r))
                for cb in range(NCC):
                    nc.tensor.matmul(
                        Aps[:, cb * H:(cb + 1) * H],
                        r(xn[:, cb * P:(cb + 1) * P]),
                        r(weights_sb[:, si * H:(si + 1) * H]),
                        start=(si == 0 and cb == 0),
                        stop=(si == NSB - 1 and cb == NCC - 1))
                nc.tensor.matmul(
                    Sps[:1, :], ones_col[:, :1],
                    r(weights_sb[:, si * H:(si + 1) * H]),
                    start=(si == 0), stop=(si == NSB - 1))
            nc.vector.tensor_copy(Asb[:], Aps[:])
            nc.vector.tensor_copy(Ssb[:1, :], Sps[:1, :])
            graw = pss.tile([P, NDB * H], f32, tag="small")
            for j in range(NDB):
                wt = wp.tile([P, D], f32r, tag="w")
                nc.sync.dma_start(
                    wt[:].rearrange("p (c m) -> p c m", c=NCC),
                    W_big[:, j * P:(j + 1) * P].rearrange("(c p) m -> p c m", p=P))
                for cc in range(NCC):
                    nc.tensor.matmul(
                        graw[:, j * H:(j + 1) * H],
                        r(wt[:, cc * P:(cc + 1) * P]),
                        r(Asb[:, cc * H:(cc + 1) * H]),
                        start=(j == 0 and cc == 0), stop=False)
                nc.tensor.matmul(
                    graw[:, j * H:(j + 1) * H],
                    r(b_row[:1, j * P:(j + 1) * P]), Ssb[:1, :],
                    start=False, stop=(j == NDB - 1))
            for j in range(NDB):
                c0 = j * H + 2 * j
                nc.vector.tensor_copy(g[0:64, j:j + 1], graw[0:64, c0:c0 + 1])
                nc.vector.tensor_copy(g[64:P, j:j + 1], graw[64:P, c0 + 1:c0 + 2])
        weighted_sum(alpha, W_qv.bitcast(f32r), bqv_row, Asb_q, Ssb_q, gq)

        # ---------- phase 5: pT = (X @ W_k + b_k) * gq, logits_b ----------
        logB = pss.tile([P, NSB * H], f32, tag="small")
        for j in range(NDB):
            wk = wp.tile([P, D], f32r, tag="w")
            nc.sync.dma_start(
                wk[:].rearrange("p (c m) -> p c m", c=NCC),
                W_k.bitcast(f32r)[:, j * P:(j + 1) * P].rearrange("(c p) m -> p c m", p=P))
            pt = ptp.tile([P, S], f32r, tag="pt")
            for sh in range(NSH):
                ps = psb.tile([P, SH], f32, tag="big")
                for cc in range(NCC):
                    nc.tensor.matmul(
                        ps[:], r(wk[:, cc * P:(cc + 1) * P]),
                        xt[:, cc * S + sh * SH: cc * S + sh * SH + SH],
                        start=(cc == 0), stop=(cc == NCC - 1))
                nc.vector.tensor_scalar(
                    pt[:, sh * SH:(sh + 1) * SH], ps[:],
                    bk_pp[:, j:j + 1], gq[:, j:j + 1], ALU.add, ALU.mult)
            for sb in range(NSB):
                nc.tensor.matmul(
                    logB[:, sb * H:(sb + 1) * H],
                    pt[:, sb * P: sb * P + P],
                    r(wks_sb[:, j * H:(j + 1) * H]),
                    start=(j == 0 and sb == 0), stop=False)
        for sb in range(NSB):
            nc.tensor.matmul(
                logB[:, sb * H:(sb + 1) * H],
                ones_row[:1, :], bks_row[:1, :],
                start=False, stop=(sb == NSB - 1))

        for sb in range(NSB):
            nc.scalar.activation(
                bE[:, sb * H:(sb + 1) * H], logB[:, sb * H:(sb + 1) * H],
                AF.Exp, bias=0.0, scale=SCALE, accum_out=Zb[:, sb:sb + 1])
        nc.vector.reciprocal(rZb[:], Zb[:])
        for sb in range(NSB):
            nc.vector.tensor_scalar(
                beta[:, sb * H:(sb + 1) * H], bE[:, sb * H:(sb + 1) * H],
                rZb[:, sb:sb + 1], None, ALU.mult)

        # ---------- phase 6: gk ----------
        weighted_sum(beta, W_k.bitcast(f32r), bk_row, Asb_k, Ssb_k, gkd)
        nc.vector.tensor_mul(gk[:], gq[:], gkd[:])

        # ---------- phase 8: out = q + (q*gk) @ W_r + b_r ----------
        # gk folded into W_r tiles: (q*gk) @ W_r == q @ (diag(gk) W_r)
        for dh in range(NDH):
            wr_tiles = []
            for cc in range(NCC):
                wr = wrp.tile([P, SH], f32r, tag=f"wrh{cc}")
                nc.sync.dma_start(
                    wr[:], W_r.bitcast(f32r)[cc * P:(cc + 1) * P, dh * SH:(dh + 1) * SH])
                nc.vector.tensor_scalar(
                    wr[:], wr[:], gk[:, cc:cc + 1], None, ALU.mult)
                wr_tiles.append(wr)
            for sb in range(NSB):
                ps = psb.tile([P, SH], f32, tag="big")
                for t in range(4):
                    j = dh * 4 + t
                    nc.tensor.matmul(
                        ps[:, t * P:(t + 1) * P].bitcast(f32r),
                        qt[:, j * S + sb * P: j * S + sb * P + P],
                        eye_r[:], is_transpose=True,
                        start=(t == 0), stop=False)
                nc.tensor.matmul(
                    ps[:], ones_row[:1, :],
                    r(br_row[:1, dh * SH:(dh + 1) * SH]),
                    start=False, stop=False)
                for cc in range(NCC):
                    nc.tensor.matmul(
                        ps[:], qt[:, cc * S + sb * P: cc * S + sb * P + P],
                        wr_tiles[cc][:],
                        start=False, stop=(cc == NCC - 1))
                ob = outp.tile([P, SH], f32, tag="ob")
                nc.scalar.copy(ob[:], ps[:])
                nc.sync.dma_start(
                    OUT[sb * P:(sb + 1) * P, dh * SH:(dh + 1) * SH], ob[:])

    nc.compile()
    return nc


def _get_nc():
    if "nc" not in _CACHE:
        _CACHE["nc"] = _build()
    return _CACHE["nc"]


def run(inputs, trace=False):
    from concourse.bass_utils import run_bass_kernel_spmd

    nc = _get_nc()
    in_maps = []
    for b in range(NCORES):
        m = {
            "X": np.ascontiguousarray(np.asarray(inputs["X"][b], dtype=np.float32)),
        }
        for k in ("W_qv", "b_qv", "W_k", "b_k", "Wq_s", "bq_s", "Wk_s", "bk_s",
                  "W_r", "b_r"):
            m[k] = np.ascontiguousarray(np.asarray(inputs[k], dtype=np.float32))
        in_maps.append(m)
    res = run_bass_kernel_spmd(nc, in_maps, core_ids=list(range(NCORES)),
                               trace=trace)
    _CACHE["last_results"] = res
    out = np.stack([res.results[b]["out"] for b in range(NCORES)], axis=0)
    return out


def kernel(**inputs):
    trace = os.environ.get("KTRACE", "0") == "1"
    return run(inputs, trace=trace)


# revision 12
# speedup vs baseline: 1.4063x; 1.4063x over previous
"""AdditiveAttention distributed Bass kernel for 8 TRN2 NeuronCores (v2, bf16).

Data-parallel over batch: B=8 samples -> 1 per core. Weights replicated.

Per-core math (S=2048, D=1024, H=16, HD=64):
  q = X @ W_qv + b_qv                 ; v = q
  k = X @ W_k + b_k
  alphas = softmax_h((q @ Wq_s + bq_s) * sc)       sc = 1/sqrt(HD)
  gq[d]  = sum_s alphas[s, h(d)] * q[s, d]         h(d) = d // 64
  p = k * gq                                        (broadcast over s)
  betas  = softmax_h((p @ Wk_s + bk_s) * sc)
  gk[d]  = gq[d] * sum_s betas[s, h(d)] * k[s, d]
  out = q + (q*gk) @ W_r + b_r

Key algebraic folds (avoid transposed-layout round trips):
  - logits_b = k @ (diag(gq) Wk_s) + bk_s          (p never materialized)
  - out      = q @ (I + diag(gk) W_r) + b_r        (residual folded into W_r)
  - gq_raw   = W_qv^T (X^T alphas) + b_qv colsum(alphas)   (q_nat never needed)

Layout: activations transposed (qT[d, s]) so big matmuls use natural weights
[c, d] as stationary and XT[c, s] as moving operand.  XT comes from the DMA
xbar transpose (bf16).  All matmuls bf16 (FWL weight loads) with f32 PSUM.
Host pre-casts X and weights to bf16.
"""

import math
import os
from contextlib import ExitStack

import numpy as np

B, S, D, H = 8, 2048, 1024, 16
HD = D // H
SCALE = 1.0 / math.sqrt(HD)
NCORES = 8
P = 128
NDB = D // P      # 8 d-blocks
NSB = S // P      # 16 s-blocks
NCC = D // P      # 8 contraction chunks
SH = 512          # psum free width for big matmuls
NSH = S // SH     # 4
NDH = D // SH     # 2

_CACHE = {}


def _build():
    import concourse.bacc as bacc
    import concourse.tile as tile
    import concourse.mybir as mybir

    f32 = mybir.dt.float32
    bf16 = mybir.dt.bfloat16
    AF = mybir.ActivationFunctionType
    ALU = mybir.AluOpType

    nc = bacc.Bacc("TRN2", target_bir_lowering=False, debug=False,
                   num_devices=NCORES)

    # bf16 inputs (host pre-cast)
    X = nc.dram_tensor("Xb", [S, D], bf16, kind="ExternalInput").ap()
    W_qv = nc.dram_tensor("W_qvb", [D, D], bf16, kind="ExternalInput").ap()
    W_k = nc.dram_tensor("W_kb", [D, D], bf16, kind="ExternalInput").ap()
    W_r = nc.dram_tensor("W_rb", [D, D], bf16, kind="ExternalInput").ap()
    Wq_s = nc.dram_tensor("Wq_sb", [D, H], bf16, kind="ExternalInput").ap()
    Wk_s = nc.dram_tensor("Wk_sb", [D, H], bf16, kind="ExternalInput").ap()
    bq_sb = nc.dram_tensor("bq_sbf", [H], bf16, kind="ExternalInput").ap()
    bk_sb = nc.dram_tensor("bk_sbf", [H], bf16, kind="ExternalInput").ap()
    br_b = nc.dram_tensor("b_rbf", [D], bf16, kind="ExternalInput").ap()
    bqv_b = nc.dram_tensor("b_qvbf", [D], bf16, kind="ExternalInput").ap()
    bkv_b = nc.dram_tensor("b_kbf", [D], bf16, kind="ExternalInput").ap()
    # f32 biases for per-partition epilogues
    b_qv = nc.dram_tensor("b_qv", [D], f32, kind="ExternalInput").ap()
    b_k = nc.dram_tensor("b_k", [D], f32, kind="ExternalInput").ap()
    OUT = nc.dram_tensor("out", [S, D], f32, kind="ExternalOutput").ap()

    with tile.TileContext(nc) as tc, ExitStack() as ctx:
        sbp = ctx.enter_context(tc.tile_pool(name="sbp", bufs=1))
        psp = ctx.enter_context(tc.tile_pool(name="psp", bufs=1, space="PSUM"))

        def st(shape, dt_, tag, bufs=1):
            return sbp.tile(shape, dt_, tag=tag, bufs=bufs, name=tag)

        def pt_(shape, tag, bufs):
            return psp.tile(shape, f32, tag=tag, bufs=bufs, name=tag)

        # ---------- constants / biases / small weights ----------
        ones_row = st([1, P], bf16, "ones_row")
        nc.gpsimd.memset(ones_row[:], 1.0)
        ones_col = st([P, 1], bf16, "ones_col")
        nc.gpsimd.memset(ones_col[:], 1.0)
        eye_bf = st([P, P], bf16, "eye_bf")
        nc.gpsimd.memset(eye_bf[:], 1.0)
        nc.gpsimd.affine_select(eye_bf[:], eye_bf[:], pattern=[[1, P]],
                                compare_op=ALU.is_equal, fill=0.0,
                                base=0, channel_multiplier=-1)

        bqv_pp = st([P, NDB], f32, "bqv_pp")
        nc.sync.dma_start(bqv_pp[:], b_qv.rearrange("(j p) -> p j", p=P))
        bk_pp = st([P, NDB], f32, "bk_pp")
        nc.sync.dma_start(bk_pp[:], b_k.rearrange("(j p) -> p j", p=P))
        bqv_row = st([1, D], bf16, "bqv_row")
        nc.sync.dma_start(bqv_row[:], bqv_b.unsqueeze(0))
        bk_row = st([1, D], bf16, "bk_row")
        nc.sync.dma_start(bk_row[:], bkv_b.unsqueeze(0))
        br_row = st([1, D], bf16, "br_row")
        nc.sync.dma_start(br_row[:], br_b.unsqueeze(0))
        bqs_row = st([1, H], bf16, "bqs_row")
        nc.sync.dma_start(bqs_row[:], bq_sb.unsqueeze(0))
        bks_row = st([1, H], bf16, "bks_row")
        nc.sync.dma_start(bks_row[:], bk_sb.unsqueeze(0))
        wqs_sb = st([P, NDB * H], bf16, "wqs_sb")
        nc.sync.dma_start(wqs_sb[:].rearrange("p (j h) -> p j h", j=NDB),
                          Wq_s.rearrange("(j p) h -> p j h", p=P))
        wks_sb = st([P, NDB * H], bf16, "wks_sb")   # becomes diag(gq)-scaled
        nc.sync.dma_start(wks_sb[:].rearrange("p (j h) -> p j h", j=NDB),
                          Wk_s.rearrange("(j p) h -> p j h", p=P))

        # ---------- resident weights (natural layout, contiguous DMA) ----------
        wqv = []
        wk = []
        wr = []
        for cc in range(NCC):
            t = st([P, D], bf16, f"wqv{cc}")
            nc.sync.dma_start(t[:], W_qv[cc * P:(cc + 1) * P, :])
            wqv.append(t)
            t = st([P, D], bf16, f"wk{cc}")
            nc.sync.dma_start(t[:], W_k[cc * P:(cc + 1) * P, :])
            wk.append(t)
            t = st([P, D], bf16, f"wr{cc}")
            nc.sync.dma_start(t[:], W_r[cc * P:(cc + 1) * P, :])
            wr.append(t)

        # ---------- small persistent intermediates ----------
        aE = st([P, NSB * H], f32, "aE")
        Za = st([P, NSB], f32, "Za")
        rZa = st([P, NSB], f32, "rZa")
        alpha = st([P, NSB * H], bf16, "alpha")
        bEx = st([P, NSB * H], f32, "bEx")
        Zb = st([P, NSB], f32, "Zb")
        rZb = st([P, NSB], f32, "rZb")
        beta = st([P, NSB * H], bf16, "beta")
        Asb_q = st([P, NCC * H], bf16, "Asbq")
        Ssb_q = st([1, H], bf16, "Ssbq")
        Asb_k = st([P, NCC * H], bf16, "Asbk")
        Ssb_k = st([1, H], bf16, "Ssbk")
        gq = st([P, NDB], f32, "gq")
        gkd = st([P, NDB], f32, "gkd")
        gk = st([P, NDB], f32, "gk")

        # ---------- big persistent activations ----------
        xt = st([P, NCC * S], bf16, "xt")   # X^T, chunk cc at cols cc*S
        qt = st([P, NDB * S], bf16, "qt")   # q^T, d-block j at cols j*S
        kt = st([P, NDB * S], bf16, "kt")   # k^T

        # ---------- phase 1: XT via DMA xbar transpose ----------
        for cc in range(NCC):
            nc.sync.dma_start_transpose(
                xt[:, cc * S:(cc + 1) * S], X[:, cc * P:(cc + 1) * P])

        # ---------- phases 2+3: qT / kT projections ----------
        def project(wtiles, bias_pp, dst):
            for j in range(NDB):
                for sh in range(NSH):
                    ps = pt_([P, SH], "big", 4)
                    for cc in range(NCC):
                        nc.tensor.matmul(
                            ps[:], wtiles[cc][:, j * P:(j + 1) * P],
                            xt[:, cc * S + sh * SH: cc * S + sh * SH + SH],
                            start=(cc == 0), stop=(cc == NCC - 1))
                    nc.vector.tensor_scalar(
                        dst[:, j * S + sh * SH: j * S + sh * SH + SH], ps[:],
                        bias_pp[:, j:j + 1], None, ALU.add)

        project(wqv, bqv_pp, qt)
        project(wk, bk_pp, kt)

        # ---------- logits + softmax (shared for alphas / betas) ----------
        def softmax_weights(src_t, w16, brow, eE, Z, rZ, wout):
            lg = pt_([P, NSB * H], "small", 3)
            for sb in range(NSB):
                for j in range(NDB):
                    nc.tensor.matmul(
                        lg[:, sb * H:(sb + 1) * H],
                        src_t[:, j * S + sb * P: j * S + sb * P + P],
                        w16[:, j * H:(j + 1) * H],
                        start=(sb == 0 and j == 0), stop=False)
                nc.tensor.matmul(
                    lg[:, sb * H:(sb + 1) * H],
                    ones_row[:1, :], brow[:1, :],
                    start=False, stop=(sb == NSB - 1))
            for sb in range(NSB):
                nc.scalar.activation(
                    eE[:, sb * H:(sb + 1) * H], lg[:, sb * H:(sb + 1) * H],
                    AF.Exp, bias=0.0, scale=SCALE, accum_out=Z[:, sb:sb + 1])
            nc.vector.reciprocal(rZ[:], Z[:])
            for sb in range(NSB):
                nc.vector.tensor_scalar(
                    wout[:, sb * H:(sb + 1) * H], eE[:, sb * H:(sb + 1) * H],
                    rZ[:, sb:sb + 1], None, ALU.mult)

        # ---------- gq_raw = W^T (X^T w) + b colsum(w), extract diagonal ----------
        def weighted_sum(weights_sb, wtiles, b_row, Asb, Ssb, g):
            Aps = pt_([P, NCC * H], "small", 3)
            Sps = pt_([1, H], "small", 3)
            for si in range(NSB):
                xn = st([P, D], bf16, "xn", bufs=3)
                nc.sync.dma_start(xn[:], X[si * P:(si + 1) * P, :])
                for cb in range(NCC):
                    nc.tensor.matmul(
                        Aps[:, cb * H:(cb + 1) * H],
                        xn[:, cb * P:(cb + 1) * P],
                        weights_sb[:, si * H:(si + 1) * H],
                        start=(si == 0 and cb == 0),
                        stop=(si == NSB - 1 and cb == NCC - 1))
                nc.tensor.matmul(
                    Sps[:1, :], ones_col[:, :1],
                    weights_sb[:, si * H:(si + 1) * H],
                    start=(si == 0), stop=(si == NSB - 1))
            nc.vector.tensor_copy(Asb[:], Aps[:])
            nc.vector.tensor_copy(Ssb[:1, :], Sps[:1, :])
            graw = pt_([P, NDB * H], "small", 3)
            for j in range(NDB):
                for cc in range(NCC):
                    nc.tensor.matmul(
                        graw[:, j * H:(j + 1) * H],
                        wtiles[cc][:, j * P:(j + 1) * P],
                        Asb[:, cc * H:(cc + 1) * H],
                        start=(j == 0 and cc == 0), stop=False)
                nc.tensor.matmul(
                    graw[:, j * H:(j + 1) * H],
                    b_row[:1, j * P:(j + 1) * P], Ssb[:1, :],
                    start=False, stop=(j == NDB - 1))
            for j in range(NDB):
                c0 = j * H + 2 * j
                nc.vector.tensor_copy(g[0:64, j:j + 1], graw[0:64, c0:c0 + 1])
                nc.vector.tensor_copy(g[64:P, j:j + 1], graw[64:P, c0 + 1:c0 + 2])

        # alphas
        softmax_weights(qt, wqs_sb, bqs_row, aE, Za, rZa, alpha)
        # gq
        weighted_sum(alpha, wqv, bqv_row, Asb_q, Ssb_q, gq)
        # fold gq into Wk_s  ->  logits_b from kT directly
        for j in range(NDB):
            nc.vector.tensor_scalar(
                wks_sb[:, j * H:(j + 1) * H], wks_sb[:, j * H:(j + 1) * H],
                gq[:, j:j + 1], None, ALU.mult)
        # betas  (logits_b = k @ (diag(gq) Wk_s) + bk_s)
        softmax_weights(kt, wks_sb, bks_row, bEx, Zb, rZb, beta)
        # gk = gq * (W_k^T (X^T beta) + b_k colsum(beta)) diag
        weighted_sum(beta, wk, bk_row, Asb_k, Ssb_k, gkd)
        nc.vector.tensor_mul(gk[:], gq[:], gkd[:])

        # ---------- fold residual + gk into W_r:  W_r' = diag(gk) W_r + I ----------
        for cc in range(NCC):
            nc.vector.tensor_scalar(
                wr[cc][:], wr[cc][:], gk[:, cc:cc + 1], None, ALU.mult)
            nc.vector.tensor_add(
                wr[cc][:, cc * P:(cc + 1) * P],
                wr[cc][:, cc * P:(cc + 1) * P], eye_bf[:])

        # ---------- final: out = q @ W_r' + b_r ----------
        for sb in range(NSB):
            for dh in range(NDH):
                ps = pt_([P, SH], "big", 4)
                nc.tensor.matmul(
                    ps[:], ones_row[:1, :],
                    br_row[:1, dh * SH:(dh + 1) * SH],
                    start=True, stop=False)
                for cc in range(NCC):
                    nc.tensor.matmul(
                        ps[:], qt[:, cc * S + sb * P: cc * S + sb * P + P],
                        wr[cc][:, dh * SH:(dh + 1) * SH],
                        start=False, stop=(cc == NCC - 1))
                ob = st([P, SH], f32, "ob", bufs=3)
                nc.scalar.copy(ob[:], ps[:])
                nc.sync.dma_start(
                    OUT[sb * P:(sb + 1) * P, dh * SH:(dh + 1) * SH], ob[:])

    nc.compile()
    return nc


def _get_nc():
    if "nc" not in _CACHE:
        _CACHE["nc"] = _build()
    return _CACHE["nc"]


def _prep_inputs(inputs):
    import ml_dtypes
    bf = ml_dtypes.bfloat16

    def f(k):
        return np.ascontiguousarray(np.asarray(inputs[k], dtype=np.float32))

    def c(a):
        return np.ascontiguousarray(np.asarray(a, dtype=np.float32).astype(bf))

    common = {
        "W_qvb": c(inputs["W_qv"]), "W_kb": c(inputs["W_k"]),
        "W_rb": c(inputs["W_r"]), "Wq_sb": c(inputs["Wq_s"]),
        "Wk_sb": c(inputs["Wk_s"]), "bq_sbf": c(inputs["bq_s"]),
        "bk_sbf": c(inputs["bk_s"]), "b_rbf": c(inputs["b_r"]),
        "b_qvbf": c(inputs["b_qv"]), "b_kbf": c(inputs["b_k"]),
        "b_qv": f("b_qv"), "b_k": f("b_k"),
    }
    in_maps = []
    for b in range(NCORES):
        m = dict(common)
        m["Xb"] = c(inputs["X"][b])
        in_maps.append(m)
    return in_maps


def run(inputs, trace=False):
    from concourse.bass_utils import run_bass_kernel_spmd

    nc = _get_nc()
    in_maps = _prep_inputs(inputs)
    res = run_bass_kernel_spmd(nc, in_maps, core_ids=list(range(NCORES)),
                               trace=trace)
    _CACHE["last_results"] = res
    out = np.stack([res.results[b]["out"] for b in range(NCORES)], axis=0)
    return out


def kernel(**inputs):
    trace = os.environ.get("KTRACE", "0") == "1"
    return run(inputs, trace=trace)


# revision 13
# speedup vs baseline: 1.6252x; 1.1557x over previous
"""AdditiveAttention distributed Bass kernel for 8 TRN2 NeuronCores (v2, bf16).

Data-parallel over batch: B=8 samples -> 1 per core. Weights replicated.

Per-core math (S=2048, D=1024, H=16, HD=64):
  q = X @ W_qv + b_qv                 ; v = q
  k = X @ W_k + b_k
  alphas = softmax_h((q @ Wq_s + bq_s) * sc)       sc = 1/sqrt(HD)
  gq[d]  = sum_s alphas[s, h(d)] * q[s, d]         h(d) = d // 64
  p = k * gq                                        (broadcast over s)
  betas  = softmax_h((p @ Wk_s + bk_s) * sc)
  gk[d]  = gq[d] * sum_s betas[s, h(d)] * k[s, d]
  out = q + (q*gk) @ W_r + b_r

Key algebraic folds (avoid transposed-layout round trips):
  - logits_b = k @ (diag(gq) Wk_s) + bk_s          (p never materialized)
  - out      = q @ (I + diag(gk) W_r) + b_r        (residual folded into W_r)
  - gq_raw   = W_qv^T (X^T alphas) + b_qv colsum(alphas)   (q_nat never needed)

Layout: activations transposed (qT[d, s]) so big matmuls use natural weights
[c, d] as stationary and XT[c, s] as moving operand.  XT comes from the DMA
xbar transpose (bf16).  All matmuls bf16 (FWL weight loads) with f32 PSUM.
Host pre-casts X and weights to bf16.
"""

import math
import os
from contextlib import ExitStack

import numpy as np

B, S, D, H = 8, 2048, 1024, 16
HD = D // H
SCALE = 1.0 / math.sqrt(HD)
NCORES = 8
P = 128
NDB = D // P      # 8 d-blocks
NSB = S // P      # 16 s-blocks
NCC = D // P      # 8 contraction chunks
SH = 512          # psum free width for big matmuls
NSH = S // SH     # 4
NDH = D // SH     # 2

_CACHE = {}


def _build():
    import concourse.bacc as bacc
    import concourse.tile as tile
    import concourse.mybir as mybir

    f32 = mybir.dt.float32
    bf16 = mybir.dt.bfloat16
    AF = mybir.ActivationFunctionType
    ALU = mybir.AluOpType

    nc = bacc.Bacc("TRN2", target_bir_lowering=False, debug=False,
                   num_devices=NCORES)

    # bf16 inputs (host pre-cast)
    X = nc.dram_tensor("Xb", [S, D], bf16, kind="ExternalInput").ap()
    W_qv = nc.dram_tensor("W_qvb", [D, D], bf16, kind="ExternalInput").ap()
    W_k = nc.dram_tensor("W_kb", [D, D], bf16, kind="ExternalInput").ap()
    W_r = nc.dram_tensor("W_rb", [D, D], bf16, kind="ExternalInput").ap()
    Wq_s = nc.dram_tensor("Wq_sb", [D, H], bf16, kind="ExternalInput").ap()
    Wk_s = nc.dram_tensor("Wk_sb", [D, H], bf16, kind="ExternalInput").ap()
    bq_sb = nc.dram_tensor("bq_sbf", [H], bf16, kind="ExternalInput").ap()
    bk_sb = nc.dram_tensor("bk_sbf", [H], bf16, kind="ExternalInput").ap()
    br_b = nc.dram_tensor("b_rbf", [D], bf16, kind="ExternalInput").ap()
    bqv_b = nc.dram_tensor("b_qvbf", [D], bf16, kind="ExternalInput").ap()
    bkv_b = nc.dram_tensor("b_kbf", [D], bf16, kind="ExternalInput").ap()
    # f32 biases for per-partition epilogues
    b_qv = nc.dram_tensor("b_qv", [D], f32, kind="ExternalInput").ap()
    b_k = nc.dram_tensor("b_k", [D], f32, kind="ExternalInput").ap()
    OUT = nc.dram_tensor("out", [S, D], f32, kind="ExternalOutput").ap()

    with tile.TileContext(nc) as tc, ExitStack() as ctx:
        sbp = ctx.enter_context(tc.tile_pool(name="sbp", bufs=1))
        psp = ctx.enter_context(tc.tile_pool(name="psp", bufs=1, space="PSUM"))

        def st(shape, dt_, tag, bufs=1):
            return sbp.tile(shape, dt_, tag=tag, bufs=bufs, name=tag)

        def pt_(shape, tag, bufs):
            return psp.tile(shape, f32, tag=tag, bufs=bufs, name=tag)

        # ---------- constants / biases / small weights ----------
        ones_row = st([1, P], bf16, "ones_row")
        nc.gpsimd.memset(ones_row[:], 1.0)
        ones_col = st([P, 1], bf16, "ones_col")
        nc.gpsimd.memset(ones_col[:], 1.0)
        eye_bf = st([P, P], bf16, "eye_bf")
        nc.gpsimd.memset(eye_bf[:], 1.0)
        nc.gpsimd.affine_select(eye_bf[:], eye_bf[:], pattern=[[1, P]],
                                compare_op=ALU.is_equal, fill=0.0,
                                base=0, channel_multiplier=-1)

        bqv_pp = st([P, NDB], f32, "bqv_pp")
        nc.sync.dma_start(bqv_pp[:], b_qv.rearrange("(j p) -> p j", p=P))
        bk_pp = st([P, NDB], f32, "bk_pp")
        nc.sync.dma_start(bk_pp[:], b_k.rearrange("(j p) -> p j", p=P))
        bqv_row = st([1, D], bf16, "bqv_row")
        nc.sync.dma_start(bqv_row[:], bqv_b.unsqueeze(0))
        bk_row = st([1, D], bf16, "bk_row")
        nc.sync.dma_start(bk_row[:], bkv_b.unsqueeze(0))
        br_row = st([1, D], bf16, "br_row")
        nc.sync.dma_start(br_row[:], br_b.unsqueeze(0))
        bqs_row = st([1, H], bf16, "bqs_row")
        nc.sync.dma_start(bqs_row[:], bq_sb.unsqueeze(0))
        bks_row = st([1, H], bf16, "bks_row")
        nc.sync.dma_start(bks_row[:], bk_sb.unsqueeze(0))
        wqs_sb = st([P, NDB * H], bf16, "wqs_sb")
        nc.sync.dma_start(wqs_sb[:].rearrange("p (j h) -> p j h", j=NDB),
                          Wq_s.rearrange("(j p) h -> p j h", p=P))
        wks_sb = st([P, NDB * H], bf16, "wks_sb")   # becomes diag(gq)-scaled
        nc.sync.dma_start(wks_sb[:].rearrange("p (j h) -> p j h", j=NDB),
                          Wk_s.rearrange("(j p) h -> p j h", p=P))

        # ---------- resident weights (natural layout, contiguous DMA) ----------
        # issue order matters: W_qv + X first (critical path), W_k next, W_r last
        wqv = []
        wk = []
        wr = []
        for cc in range(NCC):
            t = st([P, D], bf16, f"wqv{cc}")
            nc.sync.dma_start(t[:], W_qv[cc * P:(cc + 1) * P, :])
            wqv.append(t)
        xnat = st([P, NSB * D], bf16, "xnat")   # natural X, s-block si at si*D
        for si in range(NSB):
            nc.sync.dma_start(xnat[:, si * D:(si + 1) * D],
                              X[si * P:(si + 1) * P, :])
        for cc in range(NCC):
            t = st([P, D], bf16, f"wk{cc}")
            nc.sync.dma_start(t[:], W_k[cc * P:(cc + 1) * P, :])
            wk.append(t)
        for cc in range(NCC):
            t = st([P, D], bf16, f"wr{cc}")
            nc.sync.dma_start(t[:], W_r[cc * P:(cc + 1) * P, :])
            wr.append(t)

        # ---------- small persistent intermediates ----------
        aE = st([P, NSB * H], f32, "aE")
        Za = st([P, NSB], f32, "Za")
        rZa = st([P, NSB], f32, "rZa")
        alpha = st([P, NSB * H], bf16, "alpha")
        bEx = st([P, NSB * H], f32, "bEx")
        Zb = st([P, NSB], f32, "Zb")
        rZb = st([P, NSB], f32, "rZb")
        beta = st([P, NSB * H], bf16, "beta")
        Asb_q = st([P, NCC * H], bf16, "Asbq")
        Ssb_q = st([1, H], bf16, "Ssbq")
        Asb_k = st([P, NCC * H], bf16, "Asbk")
        Ssb_k = st([1, H], bf16, "Ssbk")
        gq = st([P, NDB], f32, "gq")
        gkd = st([P, NDB], f32, "gkd")
        gk = st([P, NDB], f32, "gk")

        # ---------- big persistent activations ----------
        xt = st([P, NCC * S], bf16, "xt")   # X^T, chunk cc at cols cc*S
        qt = st([P, NDB * S], bf16, "qt")   # q^T, d-block j at cols j*S
        kt = st([P, NDB * S], bf16, "kt")   # k^T

        # ---------- phase 1: XT via DMA xbar transpose (split for fine deps) ----------
        for sh in range(NSH):
            for cc in range(NCC):
                nc.sync.dma_start_transpose(
                    xt[:, cc * S + sh * SH: cc * S + (sh) * SH + SH],
                    X[sh * SH:(sh + 1) * SH, cc * P:(cc + 1) * P])

        # ---------- phases 2+3: qT / kT projections ----------
        def project(wtiles, bias_pp, dst):
            for j in range(NDB):
                for sh in range(NSH):
                    ps = pt_([P, SH], "big", 4)
                    for cc in range(NCC):
                        nc.tensor.matmul(
                            ps[:], wtiles[cc][:, j * P:(j + 1) * P],
                            xt[:, cc * S + sh * SH: cc * S + sh * SH + SH],
                            start=(cc == 0), stop=(cc == NCC - 1))
                    nc.vector.tensor_scalar(
                        dst[:, j * S + sh * SH: j * S + sh * SH + SH], ps[:],
                        bias_pp[:, j:j + 1], None, ALU.add)

        project(wqv, bqv_pp, qt)

        # ---------- logits + softmax (shared for alphas / betas) ----------
        def softmax_weights(src_t, w16, brow, eE, Z, rZ, wout, pe_filler=None):
            lg = pt_([P, NSB * H], "small", 3)
            for sb in range(NSB):
                for j in range(NDB):
                    nc.tensor.matmul(
                        lg[:, sb * H:(sb + 1) * H],
                        src_t[:, j * S + sb * P: j * S + sb * P + P],
                        w16[:, j * H:(j + 1) * H],
                        start=(sb == 0 and j == 0), stop=False)
                nc.tensor.matmul(
                    lg[:, sb * H:(sb + 1) * H],
                    ones_row[:1, :], brow[:1, :],
                    start=False, stop=(sb == NSB - 1))
            if pe_filler is not None:
                pe_filler()
            for sb in range(NSB):
                nc.scalar.activation(
                    eE[:, sb * H:(sb + 1) * H], lg[:, sb * H:(sb + 1) * H],
                    AF.Exp, bias=0.0, scale=SCALE, accum_out=Z[:, sb:sb + 1])
            nc.vector.reciprocal(rZ[:], Z[:])
            for sb in range(NSB):
                nc.vector.tensor_scalar(
                    wout[:, sb * H:(sb + 1) * H], eE[:, sb * H:(sb + 1) * H],
                    rZ[:, sb:sb + 1], None, ALU.mult)

        # ---------- gq_raw = W^T (X^T w) + b colsum(w), extract diagonal ----------
        def weighted_sum(weights_sb, wtiles, b_row, Asb, Ssb, g):
            Aps = pt_([P, NCC * H], "small", 3)
            Sps = pt_([1, H], "small", 3)
            for si in range(NSB):
                for cb in range(NCC):
                    nc.tensor.matmul(
                        Aps[:, cb * H:(cb + 1) * H],
                        xnat[:, si * D + cb * P: si * D + cb * P + P],
                        weights_sb[:, si * H:(si + 1) * H],
                        start=(si == 0 and cb == 0),
                        stop=(si == NSB - 1 and cb == NCC - 1))
                nc.tensor.matmul(
                    Sps[:1, :], ones_col[:, :1],
                    weights_sb[:, si * H:(si + 1) * H],
                    start=(si == 0), stop=(si == NSB - 1))
            nc.vector.tensor_copy(Asb[:], Aps[:])
            nc.vector.tensor_copy(Ssb[:1, :], Sps[:1, :])
            graw = pt_([P, NDB * H], "small", 3)
            for j in range(NDB):
                for cc in range(NCC):
                    nc.tensor.matmul(
                        graw[:, j * H:(j + 1) * H],
                        wtiles[cc][:, j * P:(j + 1) * P],
                        Asb[:, cc * H:(cc + 1) * H],
                        start=(j == 0 and cc == 0), stop=False)
                nc.tensor.matmul(
                    graw[:, j * H:(j + 1) * H],
                    b_row[:1, j * P:(j + 1) * P], Ssb[:1, :],
                    start=False, stop=(j == NDB - 1))
            for j in range(NDB):
                c0 = j * H + 2 * j
                nc.vector.tensor_copy(g[0:64, j:j + 1], graw[0:64, c0:c0 + 1])
                nc.vector.tensor_copy(g[64:P, j:j + 1], graw[64:P, c0 + 1:c0 + 2])

        # alphas (k-projection emitted between logits and exp so the PE has
        # dense work while ACT/DVE run the softmax tail)
        softmax_weights(qt, wqs_sb, bqs_row, aE, Za, rZa, alpha,
                        pe_filler=lambda: project(wk, bk_pp, kt))
        # gq
        weighted_sum(alpha, wqv, bqv_row, Asb_q, Ssb_q, gq)
        # fold gq into Wk_s  ->  logits_b from kT directly
        for j in range(NDB):
            nc.vector.tensor_scalar(
                wks_sb[:, j * H:(j + 1) * H], wks_sb[:, j * H:(j + 1) * H],
                gq[:, j:j + 1], None, ALU.mult)
        # betas  (logits_b = k @ (diag(gq) Wk_s) + bk_s)
        softmax_weights(kt, wks_sb, bks_row, bEx, Zb, rZb, beta)
        # gk = gq * (W_k^T (X^T beta) + b_k colsum(beta)) diag
        weighted_sum(beta, wk, bk_row, Asb_k, Ssb_k, gkd)
        nc.vector.tensor_mul(gk[:], gq[:], gkd[:])

        # ---------- fold residual + gk into W_r:  W_r' = diag(gk) W_r + I ----------
        for cc in range(NCC):
            nc.vector.tensor_scalar(
                wr[cc][:], wr[cc][:], gk[:, cc:cc + 1], None, ALU.mult)
            nc.vector.tensor_add(
                wr[cc][:, cc * P:(cc + 1) * P],
                wr[cc][:, cc * P:(cc + 1) * P], eye_bf[:])

        # ---------- final: out = q @ W_r' + b_r ----------
        for sb in range(NSB):
            for dh in range(NDH):
                ps = pt_([P, SH], "big", 4)
                nc.tensor.matmul(
                    ps[:], ones_row[:1, :],
                    br_row[:1, dh * SH:(dh + 1) * SH],
                    start=True, stop=False)
                for cc in range(NCC):
                    nc.tensor.matmul(
                        ps[:], qt[:, cc * S + sb * P: cc * S + sb * P + P],
                        wr[cc][:, dh * SH:(dh + 1) * SH],
                        start=False, stop=(cc == NCC - 1))
                ob = st([P, SH], f32, "ob", bufs=3)
                nc.scalar.copy(ob[:], ps[:])
                nc.sync.dma_start(
                    OUT[sb * P:(sb + 1) * P, dh * SH:(dh + 1) * SH], ob[:])

    nc.compile()
    return nc


def _get_nc():
    if "nc" not in _CACHE:
        _CACHE["nc"] = _build()
    return _CACHE["nc"]


def _prep_inputs(inputs):
    import ml_dtypes
    bf = ml_dtypes.bfloat16

    def f(k):
        return np.ascontiguousarray(np.asarray(inputs[k], dtype=np.float32))

    def c(a):
        return np.ascontiguousarray(np.asarray(a, dtype=np.float32).astype(bf))

    common = {
        "W_qvb": c(inputs["W_qv"]), "W_kb": c(inputs["W_k"]),
        "W_rb": c(inputs["W_r"]), "Wq_sb": c(inputs["Wq_s"]),
        "Wk_sb": c(inputs["Wk_s"]), "bq_sbf": c(inputs["bq_s"]),
        "bk_sbf": c(inputs["bk_s"]), "b_rbf": c(inputs["b_r"]),
        "b_qvbf": c(inputs["b_qv"]), "b_kbf": c(inputs["b_k"]),
        "b_qv": f("b_qv"), "b_k": f("b_k"),
    }
    in_maps = []
    for b in range(NCORES):
        m = dict(common)
        m["Xb"] = c(inputs["X"][b])
        in_maps.append(m)
    return in_maps


def run(inputs, trace=False):
    from concourse.bass_utils import run_bass_kernel_spmd

    nc = _get_nc()
    in_maps = _prep_inputs(inputs)
    res = run_bass_kernel_spmd(nc, in_maps, core_ids=list(range(NCORES)),
                               trace=trace)
    _CACHE["last_results"] = res
    out = np.stack([res.results[b]["out"] for b in range(NCORES)], axis=0)
    return out


def kernel(**inputs):
    trace = os.environ.get("KTRACE", "0") == "1"
    return run(inputs, trace=trace)


# revision 15
# speedup vs baseline: 1.7295x; 1.0642x over previous
"""AdditiveAttention distributed Bass kernel for 8 TRN2 NeuronCores (v2, bf16).

Data-parallel over batch: B=8 samples -> 1 per core. Weights replicated.

Per-core math (S=2048, D=1024, H=16, HD=64):
  q = X @ W_qv + b_qv                 ; v = q
  k = X @ W_k + b_k
  alphas = softmax_h((q @ Wq_s + bq_s) * sc)       sc = 1/sqrt(HD)
  gq[d]  = sum_s alphas[s, h(d)] * q[s, d]         h(d) = d // 64
  p = k * gq                                        (broadcast over s)
  betas  = softmax_h((p @ Wk_s + bk_s) * sc)
  gk[d]  = gq[d] * sum_s betas[s, h(d)] * k[s, d]
  out = q + (q*gk) @ W_r + b_r

Key algebraic folds (avoid transposed-layout round trips):
  - logits_b = k @ (diag(gq) Wk_s) + bk_s          (p never materialized)
  - out      = q @ (I + diag(gk) W_r) + b_r        (residual folded into W_r)
  - gq_raw   = W_qv^T (X^T alphas) + b_qv colsum(alphas)   (q_nat never needed)

Layout: activations transposed (qT[d, s]) so big matmuls use natural weights
[c, d] as stationary and XT[c, s] as moving operand.  XT comes from the DMA
xbar transpose (bf16).  All matmuls bf16 (FWL weight loads) with f32 PSUM.
Host pre-casts X and weights to bf16.
"""

import math
import os
from contextlib import ExitStack

import numpy as np

B, S, D, H = 8, 2048, 1024, 16
HD = D // H
SCALE = 1.0 / math.sqrt(HD)
NCORES = 8
P = 128
NDB = D // P      # 8 d-blocks
NSB = S // P      # 16 s-blocks
NCC = D // P      # 8 contraction chunks
SH = 512          # psum free width for big matmuls
NSH = S // SH     # 4
NDH = D // SH     # 2

_CACHE = {}


def _build():
    import concourse.bacc as bacc
    import concourse.tile as tile
    import concourse.mybir as mybir

    f32 = mybir.dt.float32
    bf16 = mybir.dt.bfloat16
    AF = mybir.ActivationFunctionType
    ALU = mybir.AluOpType

    nc = bacc.Bacc("TRN2", target_bir_lowering=False, debug=False,
                   num_devices=NCORES)

    # bf16 inputs (host pre-cast)
    X = nc.dram_tensor("Xb", [S, D], bf16, kind="ExternalInput").ap()
    W_qv = nc.dram_tensor("W_qvb", [D, D], bf16, kind="ExternalInput").ap()
    W_k = nc.dram_tensor("W_kb", [D, D], bf16, kind="ExternalInput").ap()
    W_r = nc.dram_tensor("W_rb", [D, D], bf16, kind="ExternalInput").ap()
    Wq_s = nc.dram_tensor("Wq_sb", [D, H], bf16, kind="ExternalInput").ap()
    Wk_s = nc.dram_tensor("Wk_sb", [D, H], bf16, kind="ExternalInput").ap()
    bq_sb = nc.dram_tensor("bq_sbf", [H], bf16, kind="ExternalInput").ap()
    bk_sb = nc.dram_tensor("bk_sbf", [H], bf16, kind="ExternalInput").ap()
    br_b = nc.dram_tensor("b_rbf", [D], bf16, kind="ExternalInput").ap()
    bqv_b = nc.dram_tensor("b_qvbf", [D], bf16, kind="ExternalInput").ap()
    bkv_b = nc.dram_tensor("b_kbf", [D], bf16, kind="ExternalInput").ap()
    # f32 biases for per-partition epilogues
    b_qv = nc.dram_tensor("b_qv", [D], f32, kind="ExternalInput").ap()
    b_k = nc.dram_tensor("b_k", [D], f32, kind="ExternalInput").ap()
    OUT = nc.dram_tensor("out", [S, D], f32, kind="ExternalOutput").ap()

    with tile.TileContext(nc) as tc, ExitStack() as ctx:
        sbp = ctx.enter_context(tc.tile_pool(name="sbp", bufs=1))
        psp = ctx.enter_context(tc.tile_pool(name="psp", bufs=1, space="PSUM"))

        def st(shape, dt_, tag, bufs=1):
            return sbp.tile(shape, dt_, tag=tag, bufs=bufs, name=tag)

        def pt_(shape, tag, bufs):
            return psp.tile(shape, f32, tag=tag, bufs=bufs, name=tag)

        # ---------- constants / biases / small weights ----------
        ones_row = st([1, P], bf16, "ones_row")
        nc.gpsimd.memset(ones_row[:], 1.0)
        ones_col = st([P, 1], bf16, "ones_col")
        nc.gpsimd.memset(ones_col[:], 1.0)
        eye_bf = st([P, P], bf16, "eye_bf")
        nc.gpsimd.memset(eye_bf[:], 1.0)
        nc.gpsimd.affine_select(eye_bf[:], eye_bf[:], pattern=[[1, P]],
                                compare_op=ALU.is_equal, fill=0.0,
                                base=0, channel_multiplier=-1)


        # ---------- small persistent intermediates ----------
        aE = st([P, NSB * H], f32, "aE")
        Za = st([P, NSB], f32, "Za")
        rZa = st([P, NSB], f32, "rZa")
        alpha = st([P, NSB * H], bf16, "alpha")
        bEx = st([P, NSB * H], f32, "bEx")
        Zb = st([P, NSB], f32, "Zb")
        rZb = st([P, NSB], f32, "rZb")
        beta = st([P, NSB * H], bf16, "beta")
        Asb_q = st([P, NCC * H], bf16, "Asbq")
        Ssb_q = st([1, H], bf16, "Ssbq")
        Asb_k = st([P, NCC * H], bf16, "Asbk")
        Ssb_k = st([1, H], bf16, "Ssbk")
        gq = st([P, NDB], f32, "gq")
        gkd = st([P, NDB], f32, "gkd")
        gk = st([P, NDB], f32, "gk")

        # ---------- big persistent activations / resident data ----------
        xt = st([P, NCC * S], bf16, "xt")   # X^T, chunk cc at cols cc*S
        qt = st([P, NDB * S], bf16, "qt")   # q^T, d-block j at cols j*S
        kt = st([P, NDB * S], bf16, "kt")   # k^T
        xnat = st([P, NSB * D], bf16, "xnat")  # natural X, s-block si at si*D
        wqv_all = st([P, NCC * D], bf16, "wqv_all")
        wk_all = st([P, NCC * D], bf16, "wk_all")
        wr_all = st([P, NCC * D], bf16, "wr_all")

        # DMA issue order = SP program order: XT(sh0) -> W_qv -> xnat ->
        # XT(rest) -> W_k -> small weights/biases -> W_r.
        for cc in range(NCC):
            nc.sync.dma_start_transpose(
                xt[:, cc * S: cc * S + SH],
                X[0:SH, cc * P:(cc + 1) * P])
        nc.sync.dma_start(
            wqv_all[:].rearrange("p (cc d) -> p cc d", cc=NCC),
            W_qv.rearrange("(cc p) d -> p cc d", p=P))
        nc.sync.dma_start(
            xnat[:].rearrange("p (si c) -> p si c", si=NSB),
            X.rearrange("(si p) c -> p si c", p=P))
        for cc in range(NCC):
            nc.sync.dma_start_transpose(
                xt[:, cc * S + SH: (cc + 1) * S],
                X[SH:S, cc * P:(cc + 1) * P])
        nc.sync.dma_start(
            wk_all[:].rearrange("p (cc d) -> p cc d", cc=NCC),
            W_k.rearrange("(cc p) d -> p cc d", p=P))

        bqv_pp = st([P, NDB], f32, "bqv_pp")
        nc.sync.dma_start(bqv_pp[:], b_qv.rearrange("(j p) -> p j", p=P))
        bk_pp = st([P, NDB], f32, "bk_pp")
        nc.sync.dma_start(bk_pp[:], b_k.rearrange("(j p) -> p j", p=P))
        bqv_row = st([1, D], bf16, "bqv_row")
        nc.sync.dma_start(bqv_row[:], bqv_b.unsqueeze(0))
        bk_row = st([1, D], bf16, "bk_row")
        nc.sync.dma_start(bk_row[:], bkv_b.unsqueeze(0))
        br_row = st([1, D], bf16, "br_row")
        nc.sync.dma_start(br_row[:], br_b.unsqueeze(0))
        bqs_row = st([1, H], bf16, "bqs_row")
        nc.sync.dma_start(bqs_row[:], bq_sb.unsqueeze(0))
        bks_row = st([1, H], bf16, "bks_row")
        nc.sync.dma_start(bks_row[:], bk_sb.unsqueeze(0))
        wqs_sb = st([P, NDB * H], bf16, "wqs_sb")
        nc.sync.dma_start(wqs_sb[:].rearrange("p (j h) -> p j h", j=NDB),
                          Wq_s.rearrange("(j p) h -> p j h", p=P))
        wks_sb = st([P, NDB * H], bf16, "wks_sb")   # becomes diag(gq)-scaled
        nc.sync.dma_start(wks_sb[:].rearrange("p (j h) -> p j h", j=NDB),
                          Wk_s.rearrange("(j p) h -> p j h", p=P))

        nc.sync.dma_start(
            wr_all[:].rearrange("p (cc d) -> p cc d", cc=NCC),
            W_r.rearrange("(cc p) d -> p cc d", p=P))

        # ---------- phases 2+3: qT / kT projections ----------
        def project(wall, bias_pp, dst):
            for j in range(NDB):
                for sh in range(NSH):
                    ps = pt_([P, SH], "big", 4)
                    for cc in range(NCC):
                        nc.tensor.matmul(
                            ps[:], wall[:, cc * D + j * P: cc * D + j * P + P],
                            xt[:, cc * S + sh * SH: cc * S + sh * SH + SH],
                            start=(cc == 0), stop=(cc == NCC - 1))
                    nc.vector.tensor_scalar(
                        dst[:, j * S + sh * SH: j * S + sh * SH + SH], ps[:],
                        bias_pp[:, j:j + 1], None, ALU.add)

        project(wqv_all, bqv_pp, qt)

        # ---------- logits + softmax (shared for alphas / betas) ----------
        def softmax_weights(src_t, w16, brow, eE, Z, rZ, wout, pe_filler=None):
            lg = pt_([P, NSB * H], "small", 3)
            for sb in range(NSB):
                for j in range(NDB):
                    nc.tensor.matmul(
                        lg[:, sb * H:(sb + 1) * H],
                        src_t[:, j * S + sb * P: j * S + sb * P + P],
                        w16[:, j * H:(j + 1) * H],
                        start=(sb == 0 and j == 0), stop=False)
                nc.tensor.matmul(
                    lg[:, sb * H:(sb + 1) * H],
                    ones_row[:1, :], brow[:1, :],
                    start=False, stop=(sb == NSB - 1))
            if pe_filler is not None:
                pe_filler()
            nc.scalar.activation(eE[:], lg[:], AF.Exp, bias=0.0, scale=SCALE)
            nc.vector.reduce_sum(
                Z[:].unsqueeze(2),
                eE[:].rearrange("p (sb h) -> p sb h", sb=NSB),
                axis=mybir.AxisListType.X)
            nc.vector.reciprocal(rZ[:], Z[:])
            nc.vector.tensor_tensor(
                wout[:].rearrange("p (sb h) -> p sb h", sb=NSB),
                eE[:].rearrange("p (sb h) -> p sb h", sb=NSB),
                rZ[:].unsqueeze(2).broadcast_to([P, NSB, H]),
                ALU.mult)

        # ---------- gq_raw = W^T (X^T w) + b colsum(w), extract diagonal ----------
        def weighted_sum(weights_sb, wall, b_row, Asb, Ssb, g):
            Aps = pt_([P, NCC * H], "small", 3)
            Sps = pt_([1, H], "small", 3)
            for si in range(NSB):
                for cb in range(NCC):
                    nc.tensor.matmul(
                        Aps[:, cb * H:(cb + 1) * H],
                        xnat[:, si * D + cb * P: si * D + cb * P + P],
                        weights_sb[:, si * H:(si + 1) * H],
                        start=(si == 0 and cb == 0),
                        stop=(si == NSB - 1 and cb == NCC - 1))
                nc.tensor.matmul(
                    Sps[:1, :], ones_col[:, :1],
                    weights_sb[:, si * H:(si + 1) * H],
                    start=(si == 0), stop=(si == NSB - 1))
            nc.vector.tensor_copy(Asb[:], Aps[:])
            nc.vector.tensor_copy(Ssb[:1, :], Sps[:1, :])
            graw = pt_([P, NDB * H], "small", 3)
            for j in range(NDB):
                for cc in range(NCC):
                    nc.tensor.matmul(
                        graw[:, j * H:(j + 1) * H],
                        wall[:, cc * D + j * P: cc * D + j * P + P],
                        Asb[:, cc * H:(cc + 1) * H],
                        start=(j == 0 and cc == 0), stop=False)
                nc.tensor.matmul(
                    graw[:, j * H:(j + 1) * H],
                    b_row[:1, j * P:(j + 1) * P], Ssb[:1, :],
                    start=False, stop=(j == NDB - 1))
            for j in range(NDB):
                c0 = j * H + 2 * j
                nc.vector.tensor_copy(g[0:64, j:j + 1], graw[0:64, c0:c0 + 1])
                nc.vector.tensor_copy(g[64:P, j:j + 1], graw[64:P, c0 + 1:c0 + 2])

        # alphas (k-projection emitted between logits and exp so the PE has
        # dense work while ACT/DVE run the softmax tail)
        softmax_weights(qt, wqs_sb, bqs_row, aE, Za, rZa, alpha,
                        pe_filler=lambda: project(wk_all, bk_pp, kt))
        # gq
        weighted_sum(alpha, wqv_all, bqv_row, Asb_q, Ssb_q, gq)
        # fold gq into Wk_s  ->  logits_b from kT directly
        for j in range(NDB):
            nc.vector.tensor_scalar(
                wks_sb[:, j * H:(j + 1) * H], wks_sb[:, j * H:(j + 1) * H],
                gq[:, j:j + 1], None, ALU.mult)
        # betas  (logits_b = k @ (diag(gq) Wk_s) + bk_s)
        softmax_weights(kt, wks_sb, bks_row, bEx, Zb, rZb, beta)
        # gk = gq * (W_k^T (X^T beta) + b_k colsum(beta)) diag
        weighted_sum(beta, wk_all, bk_row, Asb_k, Ssb_k, gkd)
        nc.vector.tensor_mul(gk[:], gq[:], gkd[:])

        # ---------- fold residual + gk into W_r:  W_r' = diag(gk) W_r + I ----------
        for cc in range(NCC):
            nc.vector.tensor_scalar(
                wr_all[:, cc * D:(cc + 1) * D], wr_all[:, cc * D:(cc + 1) * D],
                gk[:, cc:cc + 1], None, ALU.mult)
            nc.vector.tensor_add(
                wr_all[:, cc * D + cc * P: cc * D + (cc + 1) * P],
                wr_all[:, cc * D + cc * P: cc * D + (cc + 1) * P], eye_bf[:])

        # ---------- final: out = q @ W_r' + b_r ----------
        for sb in range(NSB):
            for dh in range(NDH):
                ps = pt_([P, SH], "big", 4)
                nc.tensor.matmul(
                    ps[:], ones_row[:1, :],
                    br_row[:1, dh * SH:(dh + 1) * SH],
                    start=True, stop=False)
                for cc in range(NCC):
                    nc.tensor.matmul(
                        ps[:], qt[:, cc * S + sb * P: cc * S + sb * P + P],
                        wr_all[:, cc * D + dh * SH: cc * D + dh * SH + SH],
                        start=False, stop=(cc == NCC - 1))
                ob = st([P, SH], f32, "ob", bufs=3)
                nc.scalar.copy(ob[:], ps[:])
                nc.sync.dma_start(
                    OUT[sb * P:(sb + 1) * P, dh * SH:(dh + 1) * SH], ob[:])

    nc.compile()
    return nc


def _get_nc():
    if "nc" not in _CACHE:
        _CACHE["nc"] = _build()
    return _CACHE["nc"]


def _prep_inputs(inputs):
    import ml_dtypes
    bf = ml_dtypes.bfloat16

    def f(k):
        return np.ascontiguousarray(np.asarray(inputs[k], dtype=np.float32))

    def c(a):
        return np.ascontiguousarray(np.asarray(a, dtype=np.float32).astype(bf))

    common = {
        "W_qvb": c(inputs["W_qv"]), "W_kb": c(inputs["W_k"]),
        "W_rb": c(inputs["W_r"]), "Wq_sb": c(inputs["Wq_s"]),
        "Wk_sb": c(inputs["Wk_s"]), "bq_sbf": c(inputs["bq_s"]),
        "bk_sbf": c(inputs["bk_s"]), "b_rbf": c(inputs["b_r"]),
        "b_qvbf": c(inputs["b_qv"]), "b_kbf": c(inputs["b_k"]),
        "b_qv": f("b_qv"), "b_k": f("b_k"),
    }
    in_maps = []
    for b in range(NCORES):
        m = dict(common)
        m["Xb"] = c(inputs["X"][b])
        in_maps.append(m)
    return in_maps


def run(inputs, trace=False):
    from concourse.bass_utils import run_bass_kernel_spmd

    nc = _get_nc()
    in_maps = _prep_inputs(inputs)
    res = run_bass_kernel_spmd(nc, in_maps, core_ids=list(range(NCORES)),
                               trace=trace)
    _CACHE["last_results"] = res
    out = np.stack([res.results[b]["out"] for b in range(NCORES)], axis=0)
    return out


def kernel(**inputs):
    trace = os.environ.get("KTRACE", "0") == "1"
    return run(inputs, trace=trace)
